# revision 22
# baseline (speedup 1.0000x reference)
"""Trainium2 Bass kernel for nn_DiagonalVariational.

out[i, d] = m[d] + sqrt(log_diag_L[d]^2 + 1e-6) * eps[i, d]

Sharding: data-parallel over the **d axis** — each of the 8 cores gets a
[2048, 2048] column slice of eps/out plus the matching [2048] slices of
m and log_diag_L. Column sharding (instead of the hinted n_sample
sharding) cuts the per-core partition-broadcast work for the [d] vectors
by 8x, which on HW is what kept n_sample-sharded variants off the DMA
roofline (gpsimd partition_broadcast measures ~2.3x slower than its
model, dominating a full-D broadcast).

Per-core kernel: partition = sample row (16 slabs of 128), free = local
d. scale = sqrt(l^2 + jitter) is computed in a [128, 16] view, staged
through a DRAM scratch into a [1, 2048] row, and broadcast across
partitions once (gpsimd). Each eps slab tile then takes two fp32
tensor_tensor ops (mul scale_b, add m_b); a few slabs' ops run on gpsimd
to keep the vector engine under the DMA roofline. Loads ride the SP
HWDGE ring, stores the ACT ring, so stores never head-of-line block
loads.
"""

import sys

sys.path.insert(0, "/opt/trn_rl_repo")

import numpy as np

D = 16384
N_SAMPLE = 2048
N_CORES = 8
D_LOCAL = D // N_CORES  # 2048
P = 128
N_SLABS = N_SAMPLE // P  # 16
JITTER = 1e-6

_CACHE = {}


def _build(
    eps_bufs=8,
    gpsimd_slabs=0,
    bcast_split=2,
    bcast_engine="dma",
    repeat=1,
    bcast_in_loop=False,
):
    import contextlib

    import concourse.bacc as bacc
    import concourse.mybir as mybir
    from concourse.tile import TileContext

    DL = D_LOCAL

    nc = bacc.Bacc("TRN2", target_bir_lowering=False, debug=False, num_devices=N_CORES)

    m_d = nc.dram_tensor("m", (DL,), mybir.dt.float32, kind="ExternalInput").ap()
    l_d = nc.dram_tensor(
        "log_diag_L", (DL,), mybir.dt.float32, kind="ExternalInput"
    ).ap()
    eps_d = nc.dram_tensor(
        "eps", (N_SAMPLE, DL), mybir.dt.float32, kind="ExternalInput"
    ).ap()
    out_d = nc.dram_tensor(
        "out", (N_SAMPLE, DL), mybir.dt.float32, kind="ExternalOutput"
    ).ap()

    with TileContext(nc) as tc:
        with (
            tc.tile_pool(name="setup", bufs=1) as setup_pool,
            tc.tile_pool(name="dram", bufs=1, space="DRAM") as dram_pool,
            tc.tile_pool(name="eps", bufs=eps_bufs) as eps_pool,
        ):
            # scale = sqrt(log_diag_L^2 + jitter) in a [128, 16] view, with
            # one Newton step (the ACT Sqrt table is only ~1e-6 relative),
            # then staged through DRAM scratch into a [1, DL] row.
            W = DL // P
            l_t = setup_pool.tile([P, W], mybir.dt.float32)
            sq_t = setup_pool.tile([P, W], mybir.dt.float32)
            scale_t = setup_pool.tile([P, W], mybir.dt.float32)
            rcp_t = setup_pool.tile([P, W], mybir.dt.float32)
            nc.sync.dma_start(out=l_t[:], in_=l_d.rearrange("(a b) -> a b", b=W))
            nc.vector.tensor_mul(out=sq_t[:], in0=l_t[:], in1=l_t[:])
            nc.vector.tensor_scalar_add(out=sq_t[:], in0=sq_t[:], scalar1=JITTER)
            nc.scalar.activation(scale_t[:], sq_t[:], mybir.ActivationFunctionType.Sqrt)
            nc.vector.reciprocal(out=rcp_t[:], in_=scale_t[:])
            nc.vector.tensor_mul(out=rcp_t[:], in0=rcp_t[:], in1=sq_t[:])
            nc.vector.tensor_add(out=scale_t[:], in0=scale_t[:], in1=rcp_t[:])
            nc.vector.tensor_scalar_mul(out=scale_t[:], in0=scale_t[:], scalar1=0.5)
            scratch = dram_pool.tile([P, W], mybir.dt.float32)
            nc.scalar.dma_start(out=scratch[:], in_=scale_t[:])
            scratch_flat = scratch[:].rearrange("a b -> (a b)")

            s_b = setup_pool.tile([P, DL], mybir.dt.float32)
            m_b = setup_pool.tile([P, DL], mybir.dt.float32)
            if bcast_engine == "gpsimd":
                s_row = setup_pool.tile([1, DL], mybir.dt.float32)
                m_row = setup_pool.tile([1, DL], mybir.dt.float32)

            def make_bcast():
                if bcast_engine == "dma":
                    # stride-0 source: every partition reads the same [DL]
                    # vector from DRAM — only ~2 MB of extra HBM reads with
                    # column sharding, and no gpsimd in the pipeline at all.
                    nc.sync.dma_start(
                        out=s_b[:], in_=scratch_flat[None, :].to_broadcast((P, DL))
                    )
                    nc.sync.dma_start(
                        out=m_b[:], in_=m_d[None, :].to_broadcast((P, DL))
                    )
                    return
                nc.sync.dma_start(out=s_row[:], in_=scratch_flat[None, :])
                nc.sync.dma_start(out=m_row[:], in_=m_d[None, :])
                # split so the first columns' broadcast (and the first TTs)
                # start sooner
                step = DL // bcast_split
                for j in range(0, DL, step):
                    js = slice(j, j + step)
                    nc.gpsimd.partition_broadcast(s_b[:, js], s_row[:, js])
                for j in range(0, DL, step):
                    js = slice(j, j + step)
                    nc.gpsimd.partition_broadcast(m_b[:, js], m_row[:, js])

            if not bcast_in_loop:
                make_bcast()

            loop_ctx = (
                tc.For_i(0, repeat, 1) if repeat > 1 else contextlib.nullcontext()
            )
            with loop_ctx:
                if bcast_in_loop:
                    make_bcast()
                # early slabs (1..gpsimd_slabs) run their elementwise ops on
                # gpsimd — early placement so the slow engine never sits on
                # the kernel tail
                gp_set = set(range(1, 1 + gpsimd_slabs))
                for s in range(N_SLABS):
                    rs = slice(s * P, (s + 1) * P)
                    t = eps_pool.tile([P, DL], mybir.dt.float32, tag="t")
                    nc.sync.dma_start(out=t[:], in_=eps_d[rs, :])
                    eng = nc.gpsimd if s in gp_set else nc.vector
                    eng.tensor_mul(out=t[:], in0=t[:], in1=s_b[:])
                    eng.tensor_add(out=t[:], in0=t[:], in1=m_b[:])
                    nc.scalar.dma_start(out=out_d[rs, :], in_=t[:])

    nc.compile()
    return nc


def _get_nc():
    if "nc" not in _CACHE:
        _CACHE["nc"] = _build()
    return _CACHE["nc"]


def _shard_inputs(m, log_diag_L, eps):
    m = np.ascontiguousarray(m, dtype=np.float32)
    log_diag_L = np.ascontiguousarray(log_diag_L, dtype=np.float32)
    eps = np.ascontiguousarray(eps, dtype=np.float32)
    return [
        {
            "m": m[i * D_LOCAL : (i + 1) * D_LOCAL],
            "log_diag_L": log_diag_L[i * D_LOCAL : (i + 1) * D_LOCAL],
            "eps": np.ascontiguousarray(eps[:, i * D_LOCAL : (i + 1) * D_LOCAL]),
        }
        for i in range(N_CORES)
    ]


def _gather_out(shards):
    return np.concatenate(list(shards), axis=1)


def kernel(m, log_diag_L, eps, **run_kwargs):
    from concourse import bass_utils

    nc = _get_nc()
    in_maps = _shard_inputs(m, log_diag_L, eps)
    res = bass_utils.run_bass_kernel_spmd(
        nc, in_maps, core_ids=list(range(N_CORES)), **run_kwargs
    )
    out = _gather_out(r["out"] for r in res.results)
    if run_kwargs:
        _CACHE["last_results"] = res
    return out
